# revision 29
# baseline (speedup 1.0000x reference)
"""Trainium2 Bass kernel for LinearAttention (B=8, S=4096, D=512, H=8, DH=64).

Sharding: data-parallel over batch -- core b processes batch element b end-to-end.

All matmul inputs are bf16 (full PE rate, no small-N penalty; rel err ~4e-3 vs
the 2e-2 gate); psum accumulates f32. x and the weights are loaded as bf16 via
gpsimd cast-DMAs (few big transfers: the SWDGE rings add ~4.5us latency each).

Per-core pipeline:
  pass A (per 512-wide s-chunk, xT prefetched one chunk ahead):
    x chunk -> PE transpose -> ACT psum drain -> xT [inner, s] bf16
    qT = Wq^T xT; phi = min(exp,1)+relu via ACT exp(+bq) / DVE relu / min / add
    k  = x Wk (+bk on DVE); phi -> Kf [s, inner];  v = x Wv (+bv on DVE)
    KV[p] += Kf[:,pair p]^T @ v'[:,p,0:129]  -- per head-pair psum accumulators,
    col 128 of v' is ones so KV's last column accumulates Ksum
  pass B (per 128-wide s-slice, eps dropped: den ~ O(1e5) >> 1e-6):
    B1: den = Qf.Ksum (block-diag rhs) -> Z=1/den -> Z^T via PE -> replicate
        across head d-partitions with the E8 selector matmul -> qfz = QfT * Zrep
    B2: O^T = KV^T @ qfz (block-diag kvsb); out = O^T(^T) Wo + bo -> DMA,
        with the Wo projection emitted one slice behind so the in-order PE
        queue never waits on the DVE/ACT divide chain.
"""

import os
import sys

import numpy as np

for _p in ("/opt/trn_rl_repo",):
    if os.path.isdir(_p) and _p not in sys.path:
        sys.path.insert(0, _p)

from contextlib import ExitStack

import concourse.bass as bass
import concourse.mybir as mybir
import concourse.tile as tile
from concourse.bass_utils import run_bass_kernel_spmd
from concourse.masks import make_identity
from concourse import library_config

B, S, D = 8, 4096, 512
H, DH = 8, 64
INNER = H * DH  # 512
EPS = 1e-6

F32 = mybir.dt.float32
BF16 = mybir.dt.bfloat16
AF = mybir.ActivationFunctionType
ALU = mybir.AluOpType

# matmul input dtype: bf16 (full-rate, no small-N penalty) or f32r
MM_DTYPE = os.environ.get("LINATTN_MM_DTYPE", "bf16")
DT_MM = BF16 if MM_DTYPE == "bf16" else mybir.dt.float32r


def _linattn_body(ctx: ExitStack, tc: "tile.TileContext", io: dict, s_total: int, reps: int = 1):
    nc = tc.nc
    NT = s_total // 128  # s-tiles
    NCH = s_total // 512  # pass-A chunks

    x_d = io["x"]
    out_d = io["out"]

    singles = ctx.enter_context(tc.tile_pool(name="singles", bufs=1))

    # ---- identity + E8 selector: FIRST on the Pool queue — anything staged
    # via gpsimd after the DMAs head-of-line blocks its DVE consumer ----
    idb = singles.tile([128, 128], DT_MM, name="idb", tag="idb")
    make_identity(nc, idb[:])
    idf = singles.tile([128, 128], F32, name="idf", tag="idf")
    make_identity(nc, idf[:])
    # E8 selector: e8[z_row, c, j] = 1 iff z_row == 2c + (j >= 64)
    e8 = singles.tile([8, 4, 128], DT_MM, name="e8", tag="e8")
    with tc.tile_pool(name="const_stage", bufs=2) as cst:
        st8 = cst.tile([8, 4, 128], F32, tag="st8")
        nc.gpsimd.memset(st8[:], 0.0)
        nc.gpsimd.affine_select(
            out=st8[:, :, 0:64], in_=st8[:, :, 0:64], compare_op=ALU.not_equal, fill=1.0,
            base=0, pattern=[[-2, 4], [0, 64]], channel_multiplier=1,
        )
        nc.gpsimd.affine_select(
            out=st8[:, :, 64:128], in_=st8[:, :, 64:128], compare_op=ALU.not_equal, fill=1.0,
            base=-1, pattern=[[-2, 4], [0, 64]], channel_multiplier=1,
        )
        nc.vector.tensor_copy(out=e8[:], in_=st8[:])

    # ---- x prefetch (gpsimd cast-DMA f32->bf16), interleaved with weights so
    # chunk-0 tiles and Wq land first on the single Pool queue. One tile per
    # 128-row slab: slab writes must not falsely serialize against transpose
    # reads of other slabs.
    x_pool = ctx.enter_context(tc.tile_pool(name="x_pool", bufs=NCH))
    x_tiles = []
    w_sb = {}

    def _dma_x(lo, hi, slabs=False):
        # big cast-DMA per 512-row chunk: SWDGE ring latency is ~4.5us per DMA
        # with only 8 in flight, so fewer/bigger transfers win in steady state.
        # The first chunks go as 4 slab-DMAs each so all 8 rings fill at once
        # and the pipeline lights up sooner.
        for ich in range(lo, min(hi, NCH)):
            t = x_pool.tile([128, 4, D], DT_MM, tag="x")
            if slabs:
                for it in range(4):
                    ist = ich * 4 + it
                    nc.gpsimd.dma_start(
                        out=t[:, it, :], in_=x_d[ist * 128 : (ist + 1) * 128, :]
                    )
            else:
                nc.gpsimd.dma_start(
                    out=t[:],
                    in_=x_d[ich * 512 : (ich + 1) * 512, :].rearrange(
                        "(it p) n -> p it n", p=128
                    ),
                )
            x_tiles.append(t)

    def _dma_w(name, slabs=False):
        t = singles.tile([128, 4, INNER], DT_MM, name=f"{name}_sb", tag=f"{name}_sb")
        if slabs:
            w3 = io[name].rearrange("(c p) n -> c p n", p=128)
            for c in range(4):
                nc.gpsimd.dma_start(out=t[:, c, :], in_=w3[c])
        else:
            nc.gpsimd.dma_start(
                out=t[:], in_=io[name].rearrange("(c p) n -> p c n", p=128)
            )  # cast f32->bf16 in-flight
        w_sb[name] = t

    # chunk 0 rides the low-latency SP HWDGE queue in f32 (the gpsimd cast
    # path has ~4.5us SWDGE latency); its transposes run in f32 instead
    x0_f32 = singles.tile([128, 4, D], F32, name="x0_f32", tag="x0_f32")
    for it0 in range(4):
        nc.sync.dma_start(
            out=x0_f32[:, it0, :], in_=x_d[it0 * 128 : (it0 + 1) * 128, :]
        )
    x_tiles.append(x0_f32)
    _dma_w("Wq")
    _dma_x(1, 2)
    # ---- biases ----
    bq_sb = singles.tile([128, 4], F32, name="bq_sb", tag="bq_sb")
    nc.gpsimd.dma_start(out=bq_sb[:], in_=io["bq"].rearrange("(c p) -> p c", p=128))
    bk_rep = singles.tile([128, INNER], F32, name="bk_rep", tag="bk_rep")
    bk_ap = io["bk"]
    nc.gpsimd.dma_start(
        out=bk_rep[:],
        in_=bass.AP(tensor=bk_ap.tensor, offset=bk_ap.offset, ap=[[0, 128]] + list(bk_ap.ap)),
    )
    bv_rep = singles.tile([128, INNER], F32, name="bv_rep", tag="bv_rep")
    bv_ap = io["bv"]
    nc.gpsimd.dma_start(
        out=bv_rep[:],
        in_=bass.AP(tensor=bv_ap.tensor, offset=bv_ap.offset, ap=[[0, 128]] + list(bv_ap.ap)),
    )
    bo_rep = singles.tile([128, D], F32, name="bo_rep", tag="bo_rep")
    bo_ap = io["bo"]
    nc.gpsimd.dma_start(
        out=bo_rep[:],
        in_=bass.AP(tensor=bo_ap.tensor, offset=bo_ap.offset, ap=[[0, 128]] + list(bo_ap.ap)),
    )
    _dma_w("Wk")
    _dma_x(2, 3)
    _dma_w("Wv")
    _dma_x(3, NCH)
    _dma_w("Wo")

    # ---- constants ----
    ones_vcol = singles.tile([128, 4, 1], DT_MM, name="ones_vcol", tag="ones_vcol")
    nc.vector.memset(ones_vcol[:], 1.0)
    # ---- persistent per-core buffers ----
    qfT = singles.tile([128, 4, s_total], DT_MM, name="qfT", tag="qfT")  # [inner, s]
    kvsb = singles.tile([128, 4, 128], DT_MM, name="kvsb", tag="kvsb")  # block-diag per pair
    # block-diag Ksum rhs: [128, pair, 8]; pair p: rows 0-63 -> col 2p, rows 64-127 -> col 2p+1
    ksum_bd = singles.tile([128, 4, 8], DT_MM, name="ksum_bd", tag="ksum_bd")
    nc.vector.memset(kvsb[:], 0.0)
    nc.vector.memset(ksum_bd[:], 0.0)

    def _passes():
        # =================== PASS A ===================
        with ExitStack() as actx:
            xT_pool = actx.enter_context(tc.tile_pool(name="xT_pool", bufs=3))
            er_pool = actx.enter_context(tc.tile_pool(name="er_pool", bufs=9))
            kf_pool = actx.enter_context(tc.tile_pool(name="kf_pool", bufs=4))
            v_pool = actx.enter_context(tc.tile_pool(name="v_pool", bufs=4))
            ps_a = actx.enter_context(tc.tile_pool(name="ps_a", bufs=4, space="PSUM"))
            ps_acc = actx.enter_context(tc.tile_pool(name="ps_acc", bufs=1, space="PSUM"))

            # KV accumulators: head pair p -> [128 d(2 heads), 128 m + ones col];
            # one psum bank each (concurrent accumulation groups need own banks)
            kvq = [
                ps_acc.tile([128, 129], F32, name=f"kvq_{p}", tag=f"kvq_{p}")[:]
                for p in range(4)
            ]

            def emit_xT(ich):
                # transpose x chunk (bf16 PE transpose ~53ns/tile, ACT drains psum);
                # DMA-xbar transposes lose: they join the DMA-ring convoy that
                # feeds x and weights, starving the PE at startup
                xT_t = xT_pool.tile([128, 4, 512], DT_MM, tag="xT")
                dt0, id0 = (F32, idf) if ich == 0 else (DT_MM, idb)
                for it in range(4):
                    xps = ps_a.tile([128, 4, 128], dt0, tag="ps")
                    for c in range(4):
                        nc.tensor.transpose(
                            xps[:, c, :],
                            x_tiles[ich][:, it, c * 128 : (c + 1) * 128],
                            id0[:],
                        )
                    nc.scalar.activation(
                        xT_t[:, :, it * 128 : (it + 1) * 128], xps[:], AF.Copy
                    )
                return xT_t

            def emit_kv(ist, kf, vq):
                for p in range(4):
                    nc.tensor.matmul(
                        kvq[p],
                        lhsT=kf[:, p * 128 : (p + 1) * 128],
                        rhs=vq[:, p, :],
                        start=(ist == 0),
                        stop=(ist == NT - 1),
                    )

            xT_t = emit_xT(0)
            for ich in range(NCH):
                # prefetch next chunk's xT so its ACT copies aren't queued
                # behind this chunk's activations
                xT_next = emit_xT(ich + 1) if ich + 1 < NCH else None
                # ---- qT = Wq^T xT; phi -> QfT ----
                for ci in range(4):
                    qps = ps_a.tile([128, 512], F32, tag="ps")
                    for cd in range(4):
                        nc.tensor.matmul(
                            qps[:],
                            lhsT=w_sb["Wq"][:, cd, ci * 128 : (ci + 1) * 128],
                            rhs=xT_t[:, cd, :],
                            start=(cd == 0),
                            stop=(cd == 3),
                        )
                    e_t = er_pool.tile([128, 512], DT_MM, tag="er")
                    r_t = er_pool.tile([128, 512], DT_MM, tag="er")
                    m_t = er_pool.tile([128, 512], DT_MM, tag="er")
                    nc.scalar.activation(e_t[:], qps[:], AF.Exp, bias=bq_sb[:, ci : ci + 1], scale=1.0)
                    nc.scalar.activation(r_t[:], qps[:], AF.Relu, bias=bq_sb[:, ci : ci + 1], scale=1.0)
                    # phi = min(exp(x),1) + relu(x); bf16 all-sbuf: 4x then 2x DVE
                    nc.vector.tensor_scalar_min(out=m_t[:], in0=e_t[:], scalar1=1.0)
                    nc.vector.tensor_add(
                        out=qfT[:, ci, ich * 512 : (ich + 1) * 512], in0=m_t[:], in1=r_t[:]
                    )
                # ---- k, v, KV accumulation per s-tile; the kvq matmuls for
                # tile it-1 are emitted after tile it's k/v matmuls so PE
                # never waits on the kf phi chain ----
                pending_kv = None
                for it in range(4):
                    ist = ich * 4 + it
                    # k (natural layout) + bias via ones-row matmul
                    kps = ps_a.tile([128, 512], F32, tag="ps")
                    for cd in range(4):
                        nc.tensor.matmul(
                            kps[:],
                            lhsT=xT_t[:, cd, it * 128 : (it + 1) * 128],
                            rhs=w_sb["Wk"][:, cd, :],
                            start=(cd == 0),
                            stop=(cd == 3),
                        )
                    # bk added on DVE (cheaper than a 512-col ones-row matmul on PE)
                    kb = kf_pool.tile([128, 512], F32, tag="kb")
                    nc.vector.tensor_add(out=kb[:], in0=kps[:], in1=bk_rep[:])
                    e_t = er_pool.tile([128, 512], DT_MM, tag="er")
                    m_t = er_pool.tile([128, 512], DT_MM, tag="er")
                    nc.scalar.activation(e_t[:], kb[:], AF.Exp)
                    nc.vector.tensor_scalar_min(out=m_t[:], in0=e_t[:], scalar1=1.0)
                    kf = kf_pool.tile([128, 512], DT_MM, tag="kf")
                    # kf = relu(k+bk) + min(exp,1)
                    nc.vector.scalar_tensor_tensor(
                        out=kf[:], in0=kb[:], scalar=0.0, in1=m_t[:], op0=ALU.max, op1=ALU.add
                    )
                    # v (natural) with bias fused into the psum->sbuf add; pair layout + ones col
                    vps = ps_a.tile([128, 512], F32, tag="ps")
                    for cd in range(4):
                        nc.tensor.matmul(
                            vps[:],
                            lhsT=xT_t[:, cd, it * 128 : (it + 1) * 128],
                            rhs=w_sb["Wv"][:, cd, :],
                            start=(cd == 0),
                            stop=(cd == 3),
                        )
                    vq = v_pool.tile([128, 4, 129], DT_MM, tag="v")
                    nc.vector.tensor_add(
                        out=vq[:, :, 0:128],
                        in0=vps[:].rearrange("p (g n) -> p g n", g=4),
                        in1=bv_rep[:].rearrange("p (g n) -> p g n", g=4),
                    )
                    nc.vector.tensor_copy(out=vq[:, :, 128:129], in_=ones_vcol[:])
                    if pending_kv is not None:
                        emit_kv(*pending_kv)
                    pending_kv = (ist, kf, vq)
                emit_kv(*pending_kv)
                xT_t = xT_next

            # ---- extract block-diag Ksum first (it gates pass B's den
            # matmuls), then the KV diag blocks (only needed by ot, later) ----
            for h in range(H):
                p, r0 = h // 2, (h % 2) * 64
                nc.vector.tensor_copy(
                    out=ksum_bd[r0 : r0 + 64, p, h : h + 1],
                    in_=kvq[p][r0 : r0 + 64, 128:129],
                )
            for h in range(H):
                p, r0 = h // 2, (h % 2) * 64
                nc.vector.tensor_copy(
                    out=kvsb[r0 : r0 + 64, p, r0 : r0 + 64],
                    in_=kvq[p][r0 : r0 + 64, r0 : r0 + 64],
                )

        # ======================= PASS B =======================
        # out_s = (Qf_s . KV_h) * Z, Z = 1/(Qf_s . Ksum_h); eps (1e-6) is
        # negligible against den ~ O(1e5) (phi > 0), so it is dropped.
        # B1 computes qfz = Qf * Z_replicated for all slices (DVE/ACT heavy,
        # little PE); B2 then streams ot -> otsb -> Wo-proj (PE heavy). The
        # in-order PE queue finishes B1's small matmuls quickly and B2's PE
        # work overlaps B1's DVE/ACT tail.
        with ExitStack() as bctx:
            dz_ps = bctx.enter_context(tc.tile_pool(name="dz_ps", bufs=2, space="PSUM"))
            zr_ps = bctx.enter_context(tc.tile_pool(name="zr_ps", bufs=2, space="PSUM"))
            ot_ps = bctx.enter_context(tc.tile_pool(name="ot_ps", bufs=2, space="PSUM"))
            ow_ps = bctx.enter_context(tc.tile_pool(name="ow_ps", bufs=2, space="PSUM"))
            zn_pool = bctx.enter_context(tc.tile_pool(name="zn_pool", bufs=6))
            zt_pool = bctx.enter_context(tc.tile_pool(name="zt_pool", bufs=6))
            zs_pool = bctx.enter_context(tc.tile_pool(name="zs_pool", bufs=6))
            qfz_pool = bctx.enter_context(tc.tile_pool(name="qfz_pool", bufs=NT))
            otsb_pool = bctx.enter_context(tc.tile_pool(name="otsb_pool", bufs=4))
            out_pool = bctx.enter_context(tc.tile_pool(name="out_pool", bufs=6))

            # ---- B1: qfz(i) for every 128-wide slice ----
            qfz_tiles = []
            for ist in range(NT):
                sl = slice(ist * 128, (ist + 1) * 128)
                den = dz_ps.tile([128, 8], F32, tag="dz")
                for c in range(4):
                    nc.tensor.matmul(
                        den[:], lhsT=qfT[:, c, sl], rhs=ksum_bd[:, c, :],
                        start=(c == 0), stop=(c == 3),
                    )
                znat = zn_pool.tile([128, 8], DT_MM, tag="zn")
                with nc.allow_low_precision(reason="Z in bf16 stays well inside tolerance"):
                    nc.vector.reciprocal(out=znat[:], in_=den[:])
                ztp = dz_ps.tile([8, 128], DT_MM, tag="dz")
                nc.tensor.transpose(ztp[:], znat[:], idb[:])
                ztsb = zt_pool.tile([8, 128], DT_MM, tag="zt")
                nc.vector.tensor_copy(out=ztsb[:], in_=ztp[:])
                # replicate Z across head d-partitions, fold into Qf
                zrep = zr_ps.tile([128, 4, 128], F32, tag="zr")
                for c in range(4):
                    nc.tensor.matmul(
                        zrep[:, c, :], lhsT=e8[:, c, :], rhs=ztsb[:], start=True, stop=True
                    )
                zrep_sb = zs_pool.tile([128, 4, 128], DT_MM, tag="zs")
                nc.scalar.activation(zrep_sb[:], zrep[:], AF.Copy)
                qfz = qfz_pool.tile([128, 4, 128], DT_MM, tag="qfz")
                nc.vector.tensor_mul(out=qfz[:], in0=qfT[:, :, sl], in1=zrep_sb[:])
                qfz_tiles.append(qfz)

            # ---- B2: O^T and the Wo projection ----
            prev_otsb = None

            def emit_ow(ist, otsb):
                owps = ow_ps.tile([128, 512], F32, tag="ow")
                for c in range(4):
                    nc.tensor.matmul(
                        owps[:],
                        lhsT=otsb[:, c, :],
                        rhs=w_sb["Wo"][:, c, :],
                        start=(c == 0),
                        stop=(c == 3),
                    )
                outt = out_pool.tile([128, 512], F32, tag="out")
                nc.vector.tensor_add(out=outt[:], in0=owps[:], in1=bo_rep[:])
                nc.sync.dma_start(out=out_d[ist * 128 : (ist + 1) * 128, :], in_=outt[:])

            for ist in range(NT):
                ot = ot_ps.tile([128, 4, 128], F32, tag="ot")
                for c in range(4):
                    nc.tensor.matmul(
                        ot[:, c, :], lhsT=kvsb[:, c, :], rhs=qfz_tiles[ist][:, c, :],
                        start=True, stop=True,
                    )
                otsb = otsb_pool.tile([128, 4, 128], DT_MM, tag="otsb")
                nc.scalar.activation(otsb[:], ot[:], AF.Copy)
                if prev_otsb is not None:
                    emit_ow(ist - 1, prev_otsb)
                prev_otsb = otsb
            emit_ow(NT - 1, prev_otsb)

    if reps == 1:
        _passes()
    else:
        with tc.For_i(0, reps, 1):
            _passes()


def _legalize_waits(nc: "bass.Bass", max_waits: int = 1) -> int:
    """This toolchain's walrus allows at most ONE sync wait per instruction.

    Tile's scheduler attaches several; hoist the extras into standalone
    event-semaphore (pure wait) instructions on the same engine, placed
    immediately before the original — identical blocking semantics since
    waits execute in stream order on the issuing sequencer.
    """
    n_split = 0
    for func in nc.m.functions:
        for block in func.blocks:
            new_insts = []
            for inst in block.instructions:
                si = getattr(inst, "sync_info", None)
                waits = list(si.on_wait) if (si and si.on_wait) else []
                if len(waits) > max_waits:
                    extra, keep = waits[:-max_waits], waits[-max_waits:]
                    for j, w in enumerate(extra):
                        ev = mybir.InstEventSemaphore(
                            name=f"{inst.name}_lw{j}",
                            engine=inst.engine,
                            ins=[],
                            outs=[],
                            sync_info=mybir.SyncInfo(on_wait=[w], on_update=[]),
                        )
                        new_insts.append(ev)
                        n_split += 1
                    si.on_wait = keep
                new_insts.append(inst)
            block.instructions[:] = new_insts
    return n_split


def build_program(s_total: int = S, reps: int = 1) -> "bass.Bass":
    nc = bass.Bass("TRN2", target_bir_lowering=False, debug=False, num_devices=B)
    io = {
        "x": nc.dram_tensor("x", [s_total, D], F32, kind="ExternalInput").ap(),
        "Wq": nc.dram_tensor("Wq", [D, INNER], F32, kind="ExternalInput").ap(),
        "bq": nc.dram_tensor("bq", [INNER], F32, kind="ExternalInput").ap(),
        "Wk": nc.dram_tensor("Wk", [D, INNER], F32, kind="ExternalInput").ap(),
        "bk": nc.dram_tensor("bk", [INNER], F32, kind="ExternalInput").ap(),
        "Wv": nc.dram_tensor("Wv", [D, INNER], F32, kind="ExternalInput").ap(),
        "bv": nc.dram_tensor("bv", [INNER], F32, kind="ExternalInput").ap(),
        "Wo": nc.dram_tensor("Wo", [INNER, D], F32, kind="ExternalInput").ap(),
        "bo": nc.dram_tensor("bo", [D], F32, kind="ExternalInput").ap(),
        "out": nc.dram_tensor("out", [s_total, D], F32, kind="ExternalOutput").ap(),
    }
    with tile.TileContext(nc) as tc:
        with ExitStack() as ctx:
            _linattn_body(ctx, tc, io, s_total, reps=reps)
    return nc


_PROGRAM_CACHE: dict = {}


def _get_program(s_total: int = S) -> "bass.Bass":
    if s_total not in _PROGRAM_CACHE:
        nc = build_program(s_total)
        _legalize_waits(nc)
        _PROGRAM_CACHE[s_total] = nc
    return _PROGRAM_CACHE[s_total]


def _in_maps(inputs: dict) -> list:
    maps = []
    for b in range(B):
        m = {"x": np.ascontiguousarray(inputs["x"][b], dtype=np.float32)}
        for name in ("Wq", "bq", "Wk", "bk", "Wv", "bv", "Wo", "bo"):
            m[name] = np.ascontiguousarray(inputs[name], dtype=np.float32)
        maps.append(m)
    return maps


def run_hw(inputs: dict, trace: bool = False, **kwargs):
    """Run on the 8 NeuronCores. Returns (out [B,S,D], BassKernelResults)."""
    nc = _get_program(S)
    res = run_bass_kernel_spmd(nc, _in_maps(inputs), list(range(B)), trace=trace, **kwargs)
    out = np.stack([res.results[b]["out"] for b in range(B)], axis=0)
    return out, res


def kernel(**inputs) -> np.ndarray:
    out, _ = run_hw(inputs, trace=False)
    return out


def bench_hw(inputs: dict, iters: int = 20, nc_override=None):
    """Time repeated NEFF executions with device-resident inputs.

    Returns (per_iter_ns, out[B,S,D] from the first run). Uses the same
    shard_map lowering as run_bass_via_pjrt, without donation so input
    buffers can be reused across timed calls.
    """
    import time as _time

    import jax
    from jax.sharding import Mesh, NamedSharding, PartitionSpec
    from jax.experimental.shard_map import shard_map

    from concourse import bass2jax
    from concourse.bass2jax import _bass_exec_p, install_neuronx_cc_hook

    install_neuronx_cc_hook()
    nc = nc_override if nc_override is not None else _get_program(S)
    in_maps = _in_maps(inputs)

    partition_name = nc.partition_id_tensor.name if nc.partition_id_tensor else None
    in_names, out_names, out_avals = [], [], []
    for alloc in nc.m.functions[0].allocations:
        if not isinstance(alloc, mybir.MemoryLocationSet):
            continue
        name = alloc.memorylocations[0].name
        if alloc.kind == "ExternalInput":
            if name != partition_name:
                in_names.append(name)
        elif alloc.kind == "ExternalOutput":
            out_names.append(name)
            out_avals.append(
                jax.core.ShapedArray(tuple(alloc.tensor_shape), mybir.dt.np(alloc.dtype))
            )
    n_params = len(in_names)
    all_in_names = in_names + out_names
    if partition_name is not None:
        all_in_names = all_in_names + [partition_name]

    def _body(*args):
        operands = list(args)
        if partition_name is not None:
            operands.append(bass2jax.partition_id_tensor())
        outs = _bass_exec_p.bind(
            *operands,
            out_avals=tuple(out_avals),
            in_names=tuple(all_in_names),
            out_names=tuple(out_names),
            lowering_input_output_aliases=(),
            sim_require_finite=True,
            sim_require_nnan=True,
            nc=nc,
        )
        return tuple(outs)

    devices = jax.devices()[:B]
    mesh = Mesh(np.asarray(devices), ("core",))
    n_outs = len(out_names)
    in_specs = (PartitionSpec("core"),) * (n_params + n_outs)
    out_specs = (PartitionSpec("core"),) * n_outs
    fn = jax.jit(
        shard_map(_body, mesh=mesh, in_specs=in_specs, out_specs=out_specs, check_rep=False)
    )

    sh = NamedSharding(mesh, PartitionSpec("core"))
    concat_in = [
        jax.device_put(
            np.concatenate([np.asarray(in_maps[c][nm])[None] for c in range(B)], axis=0).reshape(
                B * np.asarray(in_maps[0][nm]).shape[0], *np.asarray(in_maps[0][nm]).shape[1:]
            ),
            sh,
        )
        for nm in in_names
    ]
    concat_zeros = [
        jax.device_put(np.zeros((B * a.shape[0], *a.shape[1:]), a.dtype), sh) for a in out_avals
    ]

    out = fn(*concat_in, *concat_zeros)
    jax.block_until_ready(out)
    first = np.asarray(out[0]).reshape(B, *out_avals[0].shape)

    def timed(f, n):
        t0 = _time.perf_counter()
        for _ in range(n):
            r = f(*concat_in, *concat_zeros)
        jax.block_until_ready(r)
        return (_time.perf_counter() - t0) / n

    timed(fn, 3)
    t = min(timed(fn, max(5, iters // 2)) for _ in range(4))
    return int(t * 1e9), first


def build_copy_program(s_total: int = S) -> "bass.Bass":
    """Same I/O signature as the real program, near-zero work: out = x."""
    nc = bass.Bass("TRN2", target_bir_lowering=False, debug=False, num_devices=B)
    io = {}
    io["x"] = nc.dram_tensor("x", [s_total, D], F32, kind="ExternalInput").ap()
    for nm, shp in (("Wq", [D, INNER]), ("bq", [INNER]), ("Wk", [D, INNER]), ("bk", [INNER]),
                    ("Wv", [D, INNER]), ("bv", [INNER]), ("Wo", [INNER, D]), ("bo", [D])):
        io[nm] = nc.dram_tensor(nm, shp, F32, kind="ExternalInput").ap()
    out_d = nc.dram_tensor("out", [s_total, D], F32, kind="ExternalOutput").ap()
    from contextlib import ExitStack as _ES
    with tile.TileContext(nc) as tc:
        with _ES() as ctx:
            pool = ctx.enter_context(tc.tile_pool(name="cp", bufs=4))
            for i in range(s_total // 128):
                t = pool.tile([128, D], F32, tag="cp")
                sl = slice(i * 128, (i + 1) * 128)
                nc.sync.dma_start(out=t[:], in_=io["x"][sl])
                nc.sync.dma_start(out=out_d[sl], in_=t[:])
    _legalize_waits(nc)
    return nc


# revision 30
# speedup vs baseline: 54.1341x; 54.1341x over previous
"""Trainium2 Bass kernel for LinearAttention (B=8, S=4096, D=512, H=8, DH=64).

Sharding: data-parallel over batch -- core b processes batch element b end-to-end.

All matmul inputs are bf16 (full PE rate, no small-N penalty; rel err ~4e-3 vs
the 2e-2 gate); psum accumulates f32. x and the weights are loaded as bf16 via
gpsimd cast-DMAs (few big transfers: the SWDGE rings add ~4.5us latency each).

Per-core pipeline:
  pass A (per 512-wide s-chunk, xT prefetched one chunk ahead):
    x chunk -> PE transpose -> ACT psum drain -> xT [inner, s] bf16
    qT = Wq^T xT; phi = min(exp,1)+relu via ACT exp(+bq) / DVE relu / min / add
    k  = x Wk (+bk on DVE); phi -> Kf [s, inner];  v = x Wv (+bv on DVE)
    KV[p] += Kf[:,pair p]^T @ v'[:,p,0:129]  -- per head-pair psum accumulators,
    col 128 of v' is ones so KV's last column accumulates Ksum
  pass B (per 128-wide s-slice, eps dropped: den ~ O(1e5) >> 1e-6):
    B1: den = Qf.Ksum (block-diag rhs) -> Z=1/den -> Z^T via PE -> replicate
        across head d-partitions with the E8 selector matmul -> qfz = QfT * Zrep
    B2: O^T = KV^T @ qfz (block-diag kvsb); out = O^T(^T) Wo + bo -> DMA,
        with the Wo projection emitted one slice behind so the in-order PE
        queue never waits on the DVE/ACT divide chain.
"""

import os
import sys

import numpy as np

for _p in ("/opt/trn_rl_repo",):
    if os.path.isdir(_p) and _p not in sys.path:
        sys.path.insert(0, _p)

from contextlib import ExitStack

import concourse.bass as bass
import concourse.mybir as mybir
import concourse.tile as tile
from concourse.bass_utils import run_bass_kernel_spmd
from concourse.masks import make_identity
from concourse import library_config

B, S, D = 8, 4096, 512
H, DH = 8, 64
INNER = H * DH  # 512
EPS = 1e-6

F32 = mybir.dt.float32
BF16 = mybir.dt.bfloat16
AF = mybir.ActivationFunctionType
ALU = mybir.AluOpType

# matmul input dtype: bf16 (full-rate, no small-N penalty) or f32r
MM_DTYPE = os.environ.get("LINATTN_MM_DTYPE", "bf16")
DT_MM = BF16 if MM_DTYPE == "bf16" else mybir.dt.float32r


def _linattn_body(ctx: ExitStack, tc: "tile.TileContext", io: dict, s_total: int, reps: int = 1):
    nc = tc.nc
    NT = s_total // 128  # s-tiles
    NCH = s_total // 512  # pass-A chunks

    x_d = io["x"]
    out_d = io["out"]

    singles = ctx.enter_context(tc.tile_pool(name="singles", bufs=1))

    # ---- identity + E8 selector: FIRST on the Pool queue — anything staged
    # via gpsimd after the DMAs head-of-line blocks its DVE consumer ----
    idf = singles.tile([128, 128], F32, name="idf", tag="idf")
    make_identity(nc, idf[:])  # chunk-0's f32 transposes need this first
    idb = singles.tile([128, 128], DT_MM, name="idb", tag="idb")
    make_identity(nc, idb[:])
    # E8 selector: e8[z_row, c, j] = 1 iff z_row == 2c + (j >= 64)
    e8 = singles.tile([8, 4, 128], DT_MM, name="e8", tag="e8")
    with tc.tile_pool(name="const_stage", bufs=2) as cst:
        st8 = cst.tile([8, 4, 128], F32, tag="st8")
        nc.gpsimd.memset(st8[:], 0.0)
        nc.gpsimd.affine_select(
            out=st8[:, :, 0:64], in_=st8[:, :, 0:64], compare_op=ALU.not_equal, fill=1.0,
            base=0, pattern=[[-2, 4], [0, 64]], channel_multiplier=1,
        )
        nc.gpsimd.affine_select(
            out=st8[:, :, 64:128], in_=st8[:, :, 64:128], compare_op=ALU.not_equal, fill=1.0,
            base=-1, pattern=[[-2, 4], [0, 64]], channel_multiplier=1,
        )
        nc.vector.tensor_copy(out=e8[:], in_=st8[:])

    # ---- x prefetch (gpsimd cast-DMA f32->bf16), interleaved with weights so
    # chunk-0 tiles and Wq land first on the single Pool queue. One tile per
    # 128-row slab: slab writes must not falsely serialize against transpose
    # reads of other slabs.
    x_pool = ctx.enter_context(tc.tile_pool(name="x_pool", bufs=NCH))
    x_tiles = []
    w_sb = {}

    def _dma_x(lo, hi, slabs=False):
        # big cast-DMA per 512-row chunk: SWDGE ring latency is ~4.5us per DMA
        # with only 8 in flight, so fewer/bigger transfers win in steady state.
        # The first chunks go as 4 slab-DMAs each so all 8 rings fill at once
        # and the pipeline lights up sooner.
        for ich in range(lo, min(hi, NCH)):
            t = x_pool.tile([128, 4, D], DT_MM, tag="x")
            if slabs:
                for it in range(4):
                    ist = ich * 4 + it
                    nc.gpsimd.dma_start(
                        out=t[:, it, :], in_=x_d[ist * 128 : (ist + 1) * 128, :]
                    )
            else:
                nc.gpsimd.dma_start(
                    out=t[:],
                    in_=x_d[ich * 512 : (ich + 1) * 512, :].rearrange(
                        "(it p) n -> p it n", p=128
                    ),
                )
            x_tiles.append(t)

    def _dma_w(name, slabs=False):
        t = singles.tile([128, 4, INNER], DT_MM, name=f"{name}_sb", tag=f"{name}_sb")
        if slabs:
            w3 = io[name].rearrange("(c p) n -> c p n", p=128)
            for c in range(4):
                nc.gpsimd.dma_start(out=t[:, c, :], in_=w3[c])
        else:
            nc.gpsimd.dma_start(
                out=t[:], in_=io[name].rearrange("(c p) n -> p c n", p=128)
            )  # cast f32->bf16 in-flight
        w_sb[name] = t

    # chunk 0 rides the low-latency SP HWDGE queue in f32 (the gpsimd cast
    # path has ~4.5us SWDGE latency); its transposes run in f32 instead
    x0_f32 = singles.tile([128, 4, D], F32, name="x0_f32", tag="x0_f32")
    for it0 in range(4):
        nc.sync.dma_start(
            out=x0_f32[:, it0, :], in_=x_d[it0 * 128 : (it0 + 1) * 128, :]
        )
    x_tiles.append(x0_f32)
    _dma_w("Wq")
    _dma_x(1, 2)
    # ---- biases ----
    bq_sb = singles.tile([128, 4], F32, name="bq_sb", tag="bq_sb")
    nc.gpsimd.dma_start(out=bq_sb[:], in_=io["bq"].rearrange("(c p) -> p c", p=128))
    bk_rep = singles.tile([128, INNER], F32, name="bk_rep", tag="bk_rep")
    bk_ap = io["bk"]
    nc.gpsimd.dma_start(
        out=bk_rep[:],
        in_=bass.AP(tensor=bk_ap.tensor, offset=bk_ap.offset, ap=[[0, 128]] + list(bk_ap.ap)),
    )
    bv_rep = singles.tile([128, INNER], F32, name="bv_rep", tag="bv_rep")
    bv_ap = io["bv"]
    nc.gpsimd.dma_start(
        out=bv_rep[:],
        in_=bass.AP(tensor=bv_ap.tensor, offset=bv_ap.offset, ap=[[0, 128]] + list(bv_ap.ap)),
    )
    bo_rep = singles.tile([128, D], F32, name="bo_rep", tag="bo_rep")
    bo_ap = io["bo"]
    nc.gpsimd.dma_start(
        out=bo_rep[:],
        in_=bass.AP(tensor=bo_ap.tensor, offset=bo_ap.offset, ap=[[0, 128]] + list(bo_ap.ap)),
    )
    _dma_w("Wk")
    _dma_x(2, 3)
    _dma_w("Wv")
    _dma_x(3, NCH)
    _dma_w("Wo")

    # ---- constants ----
    ones_vcol = singles.tile([128, 4, 1], DT_MM, name="ones_vcol", tag="ones_vcol")
    nc.vector.memset(ones_vcol[:], 1.0)
    # ---- persistent per-core buffers ----
    qfT = singles.tile([128, 4, s_total], DT_MM, name="qfT", tag="qfT")  # [inner, s]
    kvsb = singles.tile([128, 4, 128], DT_MM, name="kvsb", tag="kvsb")  # block-diag per pair
    # block-diag Ksum rhs: [128, pair, 8]; pair p: rows 0-63 -> col 2p, rows 64-127 -> col 2p+1
    ksum_bd = singles.tile([128, 4, 8], DT_MM, name="ksum_bd", tag="ksum_bd")
    nc.vector.memset(kvsb[:], 0.0)
    nc.vector.memset(ksum_bd[:], 0.0)

    def _passes():
        # =================== PASS A ===================
        with ExitStack() as actx:
            xT_pool = actx.enter_context(tc.tile_pool(name="xT_pool", bufs=3))
            er_pool = actx.enter_context(tc.tile_pool(name="er_pool", bufs=9))
            kf_pool = actx.enter_context(tc.tile_pool(name="kf_pool", bufs=4))
            v_pool = actx.enter_context(tc.tile_pool(name="v_pool", bufs=4))
            ps_a = actx.enter_context(tc.tile_pool(name="ps_a", bufs=4, space="PSUM"))
            ps_acc = actx.enter_context(tc.tile_pool(name="ps_acc", bufs=1, space="PSUM"))

            # KV accumulators: head pair p -> [128 d(2 heads), 128 m + ones col];
            # one psum bank each (concurrent accumulation groups need own banks)
            kvq = [
                ps_acc.tile([128, 129], F32, name=f"kvq_{p}", tag=f"kvq_{p}")[:]
                for p in range(4)
            ]

            def emit_xT(ich):
                # transpose x chunk (bf16 PE transpose ~53ns/tile, ACT drains psum);
                # DMA-xbar transposes lose: they join the DMA-ring convoy that
                # feeds x and weights, starving the PE at startup
                xT_t = xT_pool.tile([128, 4, 512], DT_MM, tag="xT")
                dt0, id0 = (F32, idf) if ich == 0 else (DT_MM, idb)
                for it in range(4):
                    xps = ps_a.tile([128, 4, 128], dt0, tag="ps")
                    for c in range(4):
                        nc.tensor.transpose(
                            xps[:, c, :],
                            x_tiles[ich][:, it, c * 128 : (c + 1) * 128],
                            id0[:],
                        )
                    nc.scalar.activation(
                        xT_t[:, :, it * 128 : (it + 1) * 128], xps[:], AF.Copy
                    )
                return xT_t

            def emit_kv(ist, kf, vq):
                for p in range(4):
                    nc.tensor.matmul(
                        kvq[p],
                        lhsT=kf[:, p * 128 : (p + 1) * 128],
                        rhs=vq[:, p, :],
                        start=(ist == 0),
                        stop=(ist == NT - 1),
                    )

            xT_t = emit_xT(0)
            for ich in range(NCH):
                # prefetch next chunk's xT so its ACT copies aren't queued
                # behind this chunk's activations
                xT_next = emit_xT(ich + 1) if ich + 1 < NCH else None
                # ---- qT = Wq^T xT; phi -> QfT ----
                for ci in range(4):
                    qps = ps_a.tile([128, 512], F32, tag="ps")
                    for cd in range(4):
                        nc.tensor.matmul(
                            qps[:],
                            lhsT=w_sb["Wq"][:, cd, ci * 128 : (ci + 1) * 128],
                            rhs=xT_t[:, cd, :],
                            start=(cd == 0),
                            stop=(cd == 3),
                        )
                    e_t = er_pool.tile([128, 512], DT_MM, tag="er")
                    r_t = er_pool.tile([128, 512], DT_MM, tag="er")
                    m_t = er_pool.tile([128, 512], DT_MM, tag="er")
                    nc.scalar.activation(e_t[:], qps[:], AF.Exp, bias=bq_sb[:, ci : ci + 1], scale=1.0)
                    nc.scalar.activation(r_t[:], qps[:], AF.Relu, bias=bq_sb[:, ci : ci + 1], scale=1.0)
                    # phi = min(exp(x),1) + relu(x); bf16 all-sbuf: 4x then 2x DVE
                    nc.vector.tensor_scalar_min(out=m_t[:], in0=e_t[:], scalar1=1.0)
                    nc.vector.tensor_add(
                        out=qfT[:, ci, ich * 512 : (ich + 1) * 512], in0=m_t[:], in1=r_t[:]
                    )
                # ---- k, v, KV accumulation per s-tile; the kvq matmuls for
                # tile it-1 are emitted after tile it's k/v matmuls so PE
                # never waits on the kf phi chain ----
                pending_kv = None
                for it in range(4):
                    ist = ich * 4 + it
                    # k (natural layout) + bias via ones-row matmul
                    kps = ps_a.tile([128, 512], F32, tag="ps")
                    for cd in range(4):
                        nc.tensor.matmul(
                            kps[:],
                            lhsT=xT_t[:, cd, it * 128 : (it + 1) * 128],
                            rhs=w_sb["Wk"][:, cd, :],
                            start=(cd == 0),
                            stop=(cd == 3),
                        )
                    # bk added on DVE (cheaper than a 512-col ones-row matmul on PE)
                    kb = kf_pool.tile([128, 512], F32, tag="kb")
                    nc.vector.tensor_add(out=kb[:], in0=kps[:], in1=bk_rep[:])
                    e_t = er_pool.tile([128, 512], DT_MM, tag="er")
                    m_t = er_pool.tile([128, 512], DT_MM, tag="er")
                    nc.scalar.activation(e_t[:], kb[:], AF.Exp)
                    nc.vector.tensor_scalar_min(out=m_t[:], in0=e_t[:], scalar1=1.0)
                    kf = kf_pool.tile([128, 512], DT_MM, tag="kf")
                    # kf = relu(k+bk) + min(exp,1)
                    nc.vector.scalar_tensor_tensor(
                        out=kf[:], in0=kb[:], scalar=0.0, in1=m_t[:], op0=ALU.max, op1=ALU.add
                    )
                    # v (natural) with bias fused into the psum->sbuf add; pair layout + ones col
                    vps = ps_a.tile([128, 512], F32, tag="ps")
                    for cd in range(4):
                        nc.tensor.matmul(
                            vps[:],
                            lhsT=xT_t[:, cd, it * 128 : (it + 1) * 128],
                            rhs=w_sb["Wv"][:, cd, :],
                            start=(cd == 0),
                            stop=(cd == 3),
                        )
                    vq = v_pool.tile([128, 4, 129], DT_MM, tag="v")
                    nc.vector.tensor_add(
                        out=vq[:, :, 0:128],
                        in0=vps[:].rearrange("p (g n) -> p g n", g=4),
                        in1=bv_rep[:].rearrange("p (g n) -> p g n", g=4),
                    )
                    nc.vector.tensor_copy(out=vq[:, :, 128:129], in_=ones_vcol[:])
                    if pending_kv is not None:
                        emit_kv(*pending_kv)
                    pending_kv = (ist, kf, vq)
                emit_kv(*pending_kv)
                xT_t = xT_next

            # ---- extract block-diag Ksum first (it gates pass B's den
            # matmuls), then the KV diag blocks (only needed by ot, later) ----
            for h in range(H):
                p, r0 = h // 2, (h % 2) * 64
                nc.vector.tensor_copy(
                    out=ksum_bd[r0 : r0 + 64, p, h : h + 1],
                    in_=kvq[p][r0 : r0 + 64, 128:129],
                )
            for h in range(H):
                p, r0 = h // 2, (h % 2) * 64
                nc.vector.tensor_copy(
                    out=kvsb[r0 : r0 + 64, p, r0 : r0 + 64],
                    in_=kvq[p][r0 : r0 + 64, r0 : r0 + 64],
                )

        # ======================= PASS B =======================
        # out_s = (Qf_s . KV_h) * Z, Z = 1/(Qf_s . Ksum_h); eps (1e-6) is
        # negligible against den ~ O(1e5) (phi > 0), so it is dropped.
        # B1 computes qfz = Qf * Z_replicated for all slices (DVE/ACT heavy,
        # little PE); B2 then streams ot -> otsb -> Wo-proj (PE heavy). The
        # in-order PE queue finishes B1's small matmuls quickly and B2's PE
        # work overlaps B1's DVE/ACT tail.
        with ExitStack() as bctx:
            dz_ps = bctx.enter_context(tc.tile_pool(name="dz_ps", bufs=2, space="PSUM"))
            zr_ps = bctx.enter_context(tc.tile_pool(name="zr_ps", bufs=2, space="PSUM"))
            ot_ps = bctx.enter_context(tc.tile_pool(name="ot_ps", bufs=2, space="PSUM"))
            ow_ps = bctx.enter_context(tc.tile_pool(name="ow_ps", bufs=2, space="PSUM"))
            zn_pool = bctx.enter_context(tc.tile_pool(name="zn_pool", bufs=6))
            zt_pool = bctx.enter_context(tc.tile_pool(name="zt_pool", bufs=6))
            zs_pool = bctx.enter_context(tc.tile_pool(name="zs_pool", bufs=6))
            qfz_pool = bctx.enter_context(tc.tile_pool(name="qfz_pool", bufs=NT))
            otsb_pool = bctx.enter_context(tc.tile_pool(name="otsb_pool", bufs=4))
            out_pool = bctx.enter_context(tc.tile_pool(name="out_pool", bufs=6))

            # ---- B1: qfz(i) for every 128-wide slice ----
            qfz_tiles = []
            for ist in range(NT):
                sl = slice(ist * 128, (ist + 1) * 128)
                den = dz_ps.tile([128, 8], F32, tag="dz")
                for c in range(4):
                    nc.tensor.matmul(
                        den[:], lhsT=qfT[:, c, sl], rhs=ksum_bd[:, c, :],
                        start=(c == 0), stop=(c == 3),
                    )
                znat = zn_pool.tile([128, 8], DT_MM, tag="zn")
                with nc.allow_low_precision(reason="Z in bf16 stays well inside tolerance"):
                    nc.vector.reciprocal(out=znat[:], in_=den[:])
                ztp = dz_ps.tile([8, 128], DT_MM, tag="dz")
                nc.tensor.transpose(ztp[:], znat[:], idb[:])
                ztsb = zt_pool.tile([8, 128], DT_MM, tag="zt")
                nc.vector.tensor_copy(out=ztsb[:], in_=ztp[:])
                # replicate Z across head d-partitions, fold into Qf
                zrep = zr_ps.tile([128, 4, 128], F32, tag="zr")
                for c in range(4):
                    nc.tensor.matmul(
                        zrep[:, c, :], lhsT=e8[:, c, :], rhs=ztsb[:], start=True, stop=True
                    )
                zrep_sb = zs_pool.tile([128, 4, 128], DT_MM, tag="zs")
                nc.scalar.activation(zrep_sb[:], zrep[:], AF.Copy)
                qfz = qfz_pool.tile([128, 4, 128], DT_MM, tag="qfz")
                nc.vector.tensor_mul(out=qfz[:], in0=qfT[:, :, sl], in1=zrep_sb[:])
                qfz_tiles.append(qfz)

            # ---- B2: O^T and the Wo projection ----
            prev_otsb = None

            def emit_ow(ist, otsb):
                owps = ow_ps.tile([128, 512], F32, tag="ow")
                for c in range(4):
                    nc.tensor.matmul(
                        owps[:],
                        lhsT=otsb[:, c, :],
                        rhs=w_sb["Wo"][:, c, :],
                        start=(c == 0),
                        stop=(c == 3),
                    )
                outt = out_pool.tile([128, 512], F32, tag="out")
                nc.vector.tensor_add(out=outt[:], in0=owps[:], in1=bo_rep[:])
                nc.sync.dma_start(out=out_d[ist * 128 : (ist + 1) * 128, :], in_=outt[:])

            for ist in range(NT):
                ot = ot_ps.tile([128, 4, 128], F32, tag="ot")
                for c in range(4):
                    nc.tensor.matmul(
                        ot[:, c, :], lhsT=kvsb[:, c, :], rhs=qfz_tiles[ist][:, c, :],
                        start=True, stop=True,
                    )
                otsb = otsb_pool.tile([128, 4, 128], DT_MM, tag="otsb")
                nc.scalar.activation(otsb[:], ot[:], AF.Copy)
                if prev_otsb is not None:
                    emit_ow(ist - 1, prev_otsb)
                prev_otsb = otsb
            emit_ow(NT - 1, prev_otsb)

    if reps == 1:
        _passes()
    else:
        with tc.For_i(0, reps, 1):
            _passes()


def _legalize_waits(nc: "bass.Bass", max_waits: int = 1) -> int:
    """This toolchain's walrus allows at most ONE sync wait per instruction.

    Tile's scheduler attaches several; hoist the extras into standalone
    event-semaphore (pure wait) instructions on the same engine, placed
    immediately before the original — identical blocking semantics since
    waits execute in stream order on the issuing sequencer.
    """
    n_split = 0
    for func in nc.m.functions:
        for block in func.blocks:
            new_insts = []
            for inst in block.instructions:
                si = getattr(inst, "sync_info", None)
                waits = list(si.on_wait) if (si and si.on_wait) else []
                if len(waits) > max_waits:
                    extra, keep = waits[:-max_waits], waits[-max_waits:]
                    for j, w in enumerate(extra):
                        ev = mybir.InstEventSemaphore(
                            name=f"{inst.name}_lw{j}",
                            engine=inst.engine,
                            ins=[],
                            outs=[],
                            sync_info=mybir.SyncInfo(on_wait=[w], on_update=[]),
                        )
                        new_insts.append(ev)
                        n_split += 1
                    si.on_wait = keep
                new_insts.append(inst)
            block.instructions[:] = new_insts
    return n_split


def build_program(s_total: int = S, reps: int = 1) -> "bass.Bass":
    nc = bass.Bass("TRN2", target_bir_lowering=False, debug=False, num_devices=B)
    io = {
        "x": nc.dram_tensor("x", [s_total, D], F32, kind="ExternalInput").ap(),
        "Wq": nc.dram_tensor("Wq", [D, INNER], F32, kind="ExternalInput").ap(),
        "bq": nc.dram_tensor("bq", [INNER], F32, kind="ExternalInput").ap(),
        "Wk": nc.dram_tensor("Wk", [D, INNER], F32, kind="ExternalInput").ap(),
        "bk": nc.dram_tensor("bk", [INNER], F32, kind="ExternalInput").ap(),
        "Wv": nc.dram_tensor("Wv", [D, INNER], F32, kind="ExternalInput").ap(),
        "bv": nc.dram_tensor("bv", [INNER], F32, kind="ExternalInput").ap(),
        "Wo": nc.dram_tensor("Wo", [INNER, D], F32, kind="ExternalInput").ap(),
        "bo": nc.dram_tensor("bo", [D], F32, kind="ExternalInput").ap(),
        "out": nc.dram_tensor("out", [s_total, D], F32, kind="ExternalOutput").ap(),
    }
    with tile.TileContext(nc) as tc:
        with ExitStack() as ctx:
            _linattn_body(ctx, tc, io, s_total, reps=reps)
    return nc


_PROGRAM_CACHE: dict = {}


def _get_program(s_total: int = S) -> "bass.Bass":
    if s_total not in _PROGRAM_CACHE:
        nc = build_program(s_total)
        _legalize_waits(nc)
        _PROGRAM_CACHE[s_total] = nc
    return _PROGRAM_CACHE[s_total]


def _in_maps(inputs: dict) -> list:
    maps = []
    for b in range(B):
        m = {"x": np.ascontiguousarray(inputs["x"][b], dtype=np.float32)}
        for name in ("Wq", "bq", "Wk", "bk", "Wv", "bv", "Wo", "bo"):
            m[name] = np.ascontiguousarray(inputs[name], dtype=np.float32)
        maps.append(m)
    return maps


def run_hw(inputs: dict, trace: bool = False, **kwargs):
    """Run on the 8 NeuronCores. Returns (out [B,S,D], BassKernelResults)."""
    nc = _get_program(S)
    res = run_bass_kernel_spmd(nc, _in_maps(inputs), list(range(B)), trace=trace, **kwargs)
    out = np.stack([res.results[b]["out"] for b in range(B)], axis=0)
    return out, res


def kernel(**inputs) -> np.ndarray:
    out, _ = run_hw(inputs, trace=False)
    return out


def bench_hw(inputs: dict, iters: int = 20, nc_override=None):
    """Time repeated NEFF executions with device-resident inputs.

    Returns (per_iter_ns, out[B,S,D] from the first run). Uses the same
    shard_map lowering as run_bass_via_pjrt, without donation so input
    buffers can be reused across timed calls.
    """
    import time as _time

    import jax
    from jax.sharding import Mesh, NamedSharding, PartitionSpec
    from jax.experimental.shard_map import shard_map

    from concourse import bass2jax
    from concourse.bass2jax import _bass_exec_p, install_neuronx_cc_hook

    install_neuronx_cc_hook()
    nc = nc_override if nc_override is not None else _get_program(S)
    in_maps = _in_maps(inputs)

    partition_name = nc.partition_id_tensor.name if nc.partition_id_tensor else None
    in_names, out_names, out_avals = [], [], []
    for alloc in nc.m.functions[0].allocations:
        if not isinstance(alloc, mybir.MemoryLocationSet):
            continue
        name = alloc.memorylocations[0].name
        if alloc.kind == "ExternalInput":
            if name != partition_name:
                in_names.append(name)
        elif alloc.kind == "ExternalOutput":
            out_names.append(name)
            out_avals.append(
                jax.core.ShapedArray(tuple(alloc.tensor_shape), mybir.dt.np(alloc.dtype))
            )
    n_params = len(in_names)
    all_in_names = in_names + out_names
    if partition_name is not None:
        all_in_names = all_in_names + [partition_name]

    def _body(*args):
        operands = list(args)
        if partition_name is not None:
            operands.append(bass2jax.partition_id_tensor())
        outs = _bass_exec_p.bind(
            *operands,
            out_avals=tuple(out_avals),
            in_names=tuple(all_in_names),
            out_names=tuple(out_names),
            lowering_input_output_aliases=(),
            sim_require_finite=True,
            sim_require_nnan=True,
            nc=nc,
        )
        return tuple(outs)

    devices = jax.devices()[:B]
    mesh = Mesh(np.asarray(devices), ("core",))
    n_outs = len(out_names)
    in_specs = (PartitionSpec("core"),) * (n_params + n_outs)
    out_specs = (PartitionSpec("core"),) * n_outs
    fn = jax.jit(
        shard_map(_body, mesh=mesh, in_specs=in_specs, out_specs=out_specs, check_rep=False)
    )

    sh = NamedSharding(mesh, PartitionSpec("core"))
    concat_in = [
        jax.device_put(
            np.concatenate([np.asarray(in_maps[c][nm])[None] for c in range(B)], axis=0).reshape(
                B * np.asarray(in_maps[0][nm]).shape[0], *np.asarray(in_maps[0][nm]).shape[1:]
            ),
            sh,
        )
        for nm in in_names
    ]
    concat_zeros = [
        jax.device_put(np.zeros((B * a.shape[0], *a.shape[1:]), a.dtype), sh) for a in out_avals
    ]

    out = fn(*concat_in, *concat_zeros)
    jax.block_until_ready(out)
    first = np.asarray(out[0]).reshape(B, *out_avals[0].shape)

    def timed(f, n):
        t0 = _time.perf_counter()
        for _ in range(n):
            r = f(*concat_in, *concat_zeros)
        jax.block_until_ready(r)
        return (_time.perf_counter() - t0) / n

    timed(fn, 3)
    t = min(timed(fn, max(5, iters // 2)) for _ in range(4))
    return int(t * 1e9), first


def build_copy_program(s_total: int = S) -> "bass.Bass":
    """Same I/O signature as the real program, near-zero work: out = x."""
    nc = bass.Bass("TRN2", target_bir_lowering=False, debug=False, num_devices=B)
    io = {}
    io["x"] = nc.dram_tensor("x", [s_total, D], F32, kind="ExternalInput").ap()
    for nm, shp in (("Wq", [D, INNER]), ("bq", [INNER]), ("Wk", [D, INNER]), ("bk", [INNER]),
                    ("Wv", [D, INNER]), ("bv", [INNER]), ("Wo", [INNER, D]), ("bo", [D])):
        io[nm] = nc.dram_tensor(nm, shp, F32, kind="ExternalInput").ap()
    out_d = nc.dram_tensor("out", [s_total, D], F32, kind="ExternalOutput").ap()
    from contextlib import ExitStack as _ES
    with tile.TileContext(nc) as tc:
        with _ES() as ctx:
            pool = ctx.enter_context(tc.tile_pool(name="cp", bufs=4))
            for i in range(s_total // 128):
                t = pool.tile([128, D], F32, tag="cp")
                sl = slice(i * 128, (i + 1) * 128)
                nc.sync.dma_start(out=t[:], in_=io["x"][sl])
                nc.sync.dma_start(out=out_d[sl], in_=t[:])
    _legalize_waits(nc)
    return nc
